# revision 22
# baseline (speedup 1.0000x reference)
"""Trainium2 Bass kernel for nn_PoseCDE.

Mathematical structure exploited (exact, input-independent):
  The CDE integrates over t in [0.1, 1.0], which lies entirely inside the
  FIRST segment of the rectilinear control path (segment grid spacing is 1,
  bucketize gives idx=0 for all eval times).  Segment 0's derivative is the
  time-advance knot: (ts[b,2]-ts[b,1], 0, ..., 0).  Hence
      f(t, z) = s_b * g(z)[:, :, 0]
  and only column 0 of each C-group of Wout matters:  Wsub = Wout[:, ::C].
  The 512 x 262656 matmul collapses to 512 x 512 (verified exact vs the
  reference for arbitrary inputs).

Device computation per core (data-parallel over batch, 8 rows per core):
  36 sequential 3-layer MLP evals (RK4, 9 steps) + linear regressor head.
  Activations are kept TRANSPOSED ([H on partitions, batch on free]) so
  weights are the PE-stationary operand and no on-chip transposes are
  needed; biases become per-partition operands.

Scheduling notes (trn2):
  - per-H-chunk PSUM groups + k-phase-major matmul order: each chunk's
    activation (DVE for early relu chunks, ACT otherwise) completes while
    the PE still streams the later chunks, so the next layer's k-phases
    find their inputs ready (software pipelining of the serial MLP chain).
  - weights in bf16 (FWL), fp32 PSUM accumulation, fp32 RK4 state.
  - uniform s and the uniform RK4 step h are folded into immediates.
"""

import os
import numpy as np
import ml_dtypes

import concourse.bass as bass
import concourse.bacc as bacc
import concourse.mybir as mybir
from concourse.tile import TileContext
from concourse.bass_utils import run_bass_kernel_spmd

N_CORES = 8
B = 64
BL = B // N_CORES          # batch rows per core
H = 512
C = H + 1
KC = H // 128              # H chunks (4)
NSLOT = 10                 # z0 + 9 RK4 states
F32 = mybir.dt.float32
BF16 = mybir.dt.bfloat16
F16 = mybir.dt.float16
NAUX = 3 * KC + 2 + BL     # packed aux columns: b0|b1|bs|br1|br2|svec


def _build_nc(hvals, wdt, fold_s, zero_bias):
    """wdt: "f32"|"bf16"|"f16"; fold_s: None or uniform-s float;
    zero_bias: True = MLP biases known to be zero."""
    nc = bacc.Bacc(None)
    WDT = {"f32": F32, "bf16": BF16, "f16": F16}[wdt]
    wdt_lp = wdt != "f32"

    w0 = nc.declare_dram_parameter("w0", [H, H], WDT, isOutput=False)
    w1 = nc.declare_dram_parameter("w1", [H, H], WDT, isOutput=False)
    ws = nc.declare_dram_parameter("ws", [H, H], WDT, isOutput=False)
    wr1 = nc.declare_dram_parameter("wr1", [H, 128], WDT, isOutput=False)
    wr2 = nc.declare_dram_parameter("wr2", [128, 6], F32, isOutput=False)
    aux = nc.declare_dram_parameter("aux", [128, NAUX], F32, isOutput=False)

    posesT = nc.declare_dram_parameter("posesT", [6, NSLOT, BL], F32, isOutput=True)
    hlastT = nc.declare_dram_parameter("hlastT", [128, KC, BL], F32, isOutput=True)

    relu = mybir.ActivationFunctionType.Relu
    tanh = mybir.ActivationFunctionType.Tanh
    ident = mybir.ActivationFunctionType.Identity
    mult = mybir.AluOpType.mult
    add = mybir.AluOpType.add
    amax = mybir.AluOpType.max

    def f32c(x):
        return float(np.float32(x))

    with TileContext(nc) as tc:
        with (
            tc.tile_pool(name="weights", bufs=1) as wpool,
            tc.tile_pool(name="state", bufs=1) as spool,
            tc.tile_pool(name="scratch", bufs=2) as scr,
            tc.tile_pool(name="psum", bufs=1, space="PSUM") as pp,
            tc.tile_pool(name="psum_r", bufs=1, space="PSUM") as ppr,
            tc.tile_pool(name="outs", bufs=1) as opool,
        ):
            # ---- load weights (parallel DMA dispatch across engines) ----
            w0sb = wpool.tile([128, KC, H], WDT, tag="w0")
            w1sb = wpool.tile([128, KC, H], WDT, tag="w1")
            wssb = wpool.tile([128, KC, H], WDT, tag="ws")
            wr1sb = wpool.tile([128, KC, 128], WDT, tag="wr1")
            wr2sb = wpool.tile([128, 6], F32, tag="wr2")
            auxsb = wpool.tile([128, NAUX], F32, tag="aux")
            nc.sync.dma_start(out=w0sb, in_=w0[:].rearrange("(k p) n -> p k n", p=128))
            nc.scalar.dma_start(out=w1sb, in_=w1[:].rearrange("(k p) n -> p k n", p=128))
            nc.gpsimd.dma_start(out=wssb, in_=ws[:].rearrange("(k p) n -> p k n", p=128))
            nc.gpsimd.dma_start(out=wr1sb, in_=wr1[:].rearrange("(k p) n -> p k n", p=128))
            nc.sync.dma_start(out=auxsb, in_=aux[:])
            nc.sync.dma_start(out=wr2sb, in_=wr2[:])

            b0sb = auxsb[:, 0:KC]
            b1sb = auxsb[:, KC:2 * KC]
            bssb = auxsb[:, 2 * KC:3 * KC]
            br1sb = auxsb[:, 3 * KC:3 * KC + 1]
            br2sb = auxsb[:, 3 * KC + 1:3 * KC + 2]   # first 6 partitions valid
            s_sb = auxsb[:, 3 * KC + 2:3 * KC + 2 + BL]

            def s_ap():
                # broadcast s_sb [128, BL] over the KC free dim
                t = s_sb
                return bass.AP(
                    tensor=t.tensor, offset=t.offset,
                    ap=[t.ap[0], [0, KC], t.ap[1]],
                )

            # ---- absorb input-DMA sems into engine vector clocks ----
            # (the S3_LW weight-load struct only fits ONE sync wait; these
            # 1x1 matmuls make every later PE inst see the DMAs as done)
            dummy_ps = ppr.tile([1, 1], F32, name="dps", tag="psr")

            def absorb(wtile):
                sl = wtile[:, 0, 0:1] if len(wtile.shape) == 3 else wtile[:, 0:1]
                nc.tensor.matmul(dummy_ps[:, :], lhsT=sl, rhs=sl,
                                 start=True, stop=True)

            absorb(w0sb)
            _pending_absorbs = [w1sb, wssb, wr1sb, wr2sb]

            # ---- state buffers ----
            zbuf = spool.tile([128, NSLOT, KC, BL], F32, tag="zbuf")
            nc.vector.memset(zbuf[:, 0], 0.0)
            if wdt_lp:
                zbuf_m = spool.tile([128, NSLOT, KC, BL], WDT, tag="zbufm")
                nc.vector.memset(zbuf_m[:, 0], 0.0)

            def mlp_layer(in_t, w_t, b_t, func, out_t):
                """out_t[128,KC,BL] = func(matmul(in_t) + bias), per-chunk
                pipelined: psum chunk c is consumed while the PE streams
                chunk c+1."""
                is_relu = func is relu
                pss = [pp.tile([128, BL], F32, name=f"psc{c}", tag=f"psc{c}")
                       for c in range(KC)]
                for k in range(KC):            # k-phase major
                    for c_ in range(KC):
                        nc.tensor.matmul(
                            pss[c_][:, :],
                            lhsT=w_t[:, k, bass.ts(c_, 128)],
                            rhs=in_t[:, k, :],
                            start=(k == 0),
                            stop=(k == KC - 1),
                            skip_group_check=True,
                        )
                for c_ in range(KC):
                    ps = pss[c_]
                    if is_relu and c_ < KC // 2:
                        if zero_bias:
                            nc.vector.tensor_scalar(
                                out_t[:, c_], ps[:, :], 0.0, None, op0=amax)
                        else:
                            nc.vector.tensor_scalar(
                                out_t[:, c_], ps[:, :],
                                b_t[:, c_:c_ + 1], 0.0, op0=add, op1=amax)
                    else:
                        bias = 0.0 if zero_bias else b_t[:, c_:c_ + 1]
                        nc.scalar.activation(
                            out_t[:, c_], ps[:, :], func, bias=bias, scale=1.0)

            def mlp(in_t, utag):
                h1 = scr.tile([128, KC, BL], WDT, tag="h1")
                h2 = scr.tile([128, KC, BL], WDT, tag="h2")
                u = scr.tile([128, KC, BL], F32, tag=utag)
                mlp_layer(in_t, w0sb, b0sb, relu, h1)
                if _pending_absorbs:
                    absorb(_pending_absorbs.pop(0))
                mlp_layer(h1, w1sb, b1sb, relu, h2)
                if _pending_absorbs:
                    absorb(_pending_absorbs.pop(0))
                mlp_layer(h2, wssb, bssb, tanh, u)
                while _pending_absorbs:
                    absorb(_pending_absorbs.pop(0))
                return u

            # ---- RK4 ----
            for t in range(9):
                hf = f32c(hvals[t])
                half_h = f32c(np.float32(0.5) * np.float32(hf))
                h6 = f32c(np.float32(hf) / np.float32(6.0))
                z_t = zbuf[:, t]

                if wdt_lp:
                    if fold_s is None and t > 0:
                        zt_m = scr.tile([128, KC, BL], WDT, tag="ztm")
                        nc.vector.tensor_copy(zt_m[:], z_t)
                        ev1_in = zt_m
                    else:
                        ev1_in = zbuf_m[:, t]
                else:
                    ev1_in = z_t

                if fold_s is not None:
                    sv = np.float32(fold_s)
                    c1 = f32c(np.float32(half_h) * sv)   # 0.5*h*s
                    c2 = f32c(np.float32(hf) * sv)       # h*s
                    c3 = f32c(np.float32(h6) * sv)       # (h/6)*s

                    def chunk_stt(dst, src0, scal, src1):
                        for c_ in range(KC):
                            nc.vector.scalar_tensor_tensor(
                                out=dst[:, c_], in0=src0[:, c_], scalar=scal,
                                in1=src1[:, c_], op0=mult, op1=add)

                    u1 = mlp(ev1_in, "u1")
                    zin1 = scr.tile([128, KC, BL], WDT, tag="zin")
                    chunk_stt(zin1, u1, c1, z_t)
                    u2 = mlp(zin1, "u2")
                    zin2 = scr.tile([128, KC, BL], WDT, tag="zin")
                    chunk_stt(zin2, u2, c1, z_t)
                    # acc = u1 + 2*u2 (+2*u3) built mid-step, off-chain
                    acc = scr.tile([128, KC, BL], F32, tag="acc")
                    nc.vector.scalar_tensor_tensor(
                        out=acc[:], in0=u2[:], scalar=2.0, in1=u1[:],
                        op0=mult, op1=add)
                    u3 = mlp(zin2, "u3")
                    zin3 = scr.tile([128, KC, BL], WDT, tag="zin")
                    chunk_stt(zin3, u3, c2, z_t)
                    nc.vector.scalar_tensor_tensor(
                        out=acc[:], in0=u3[:], scalar=2.0, in1=acc[:],
                        op0=mult, op1=add)
                    # w = z + c3*acc  (so z' = w + c3*u4: kills one serial
                    # DVE hop at the step boundary)
                    wv = scr.tile([128, KC, BL], F32, tag="wv")
                    nc.vector.scalar_tensor_tensor(
                        out=wv[:], in0=acc[:], scalar=c3, in1=z_t[:],
                        op0=mult, op1=add)
                    u4 = mlp(zin3, "u4")
                    if wdt_lp:
                        chunk_stt(zbuf_m[:, t + 1], u4, c3, wv)
                    nc.vector.scalar_tensor_tensor(
                        out=zbuf[:, t + 1], in0=u4[:], scalar=c3, in1=wv[:],
                        op0=mult, op1=add)
                else:
                    u1 = mlp(ev1_in, "u1")
                    k1 = scr.tile([128, KC, BL], F32, tag="k1")
                    nc.vector.tensor_mul(k1[:], u1[:], s_ap())
                    zin1 = scr.tile([128, KC, BL], WDT, tag="zin")
                    nc.vector.scalar_tensor_tensor(
                        out=zin1[:], in0=k1[:], scalar=half_h, in1=z_t,
                        op0=mult, op1=add)
                    u2 = mlp(zin1, "u2")
                    k2 = scr.tile([128, KC, BL], F32, tag="k2")
                    nc.vector.tensor_mul(k2[:], u2[:], s_ap())
                    zin2 = scr.tile([128, KC, BL], WDT, tag="zin")
                    nc.vector.scalar_tensor_tensor(
                        out=zin2[:], in0=k2[:], scalar=half_h, in1=z_t,
                        op0=mult, op1=add)
                    u3 = mlp(zin2, "u3")
                    k3 = scr.tile([128, KC, BL], F32, tag="k3")
                    nc.vector.tensor_mul(k3[:], u3[:], s_ap())
                    zin3 = scr.tile([128, KC, BL], WDT, tag="zin")
                    nc.vector.scalar_tensor_tensor(
                        out=zin3[:], in0=k3[:], scalar=hf, in1=z_t,
                        op0=mult, op1=add)
                    u4 = mlp(zin3, "u4")
                    k4 = scr.tile([128, KC, BL], F32, tag="k4")
                    nc.vector.tensor_mul(k4[:], u4[:], s_ap())
                    acc = scr.tile([128, KC, BL], F32, tag="acc")
                    nc.vector.scalar_tensor_tensor(
                        out=acc[:], in0=k2[:], scalar=2.0, in1=k1[:],
                        op0=mult, op1=add)
                    nc.vector.scalar_tensor_tensor(
                        out=acc[:], in0=k3[:], scalar=2.0, in1=acc[:],
                        op0=mult, op1=add)
                    nc.vector.tensor_add(acc[:], acc[:], k4[:])
                    nc.vector.scalar_tensor_tensor(
                        out=zbuf[:, t + 1], in0=acc[:], scalar=h6, in1=z_t,
                        op0=mult, op1=add)

            # ---- regressor head ----
            if wdt_lp and fold_s is not None:
                zreg = zbuf_m
            elif wdt_lp:
                zreg = spool.tile([128, NSLOT, KC, BL], WDT, tag="zregm")
                nc.vector.tensor_copy(zreg[:], zbuf[:])
            else:
                zreg = zbuf
            ps_r = ppr.tile([128, NSLOT * BL], F32, tag="psr2")
            for k in range(KC):
                nc.tensor.matmul(
                    ps_r[:, :],
                    lhsT=wr1sb[:, k, :],
                    rhs=zreg[:, :, k, :],
                    start=(k == 0),
                    stop=(k == KC - 1),
                )
            # leaky_relu(x) = max(x, 0.1*x)  (exact for slope < 1)
            xr = opool.tile([128, NSLOT * BL], F32, tag="xr")
            nc.scalar.activation(
                xr[:, :], ps_r[:, :], ident,
                bias=br1sb, scale=1.0,
            )
            xs = opool.tile([128, NSLOT * BL], F32, tag="xs")
            nc.vector.tensor_scalar_mul(xs[:, :], xr[:, :], 0.1)
            hr = opool.tile([128, NSLOT * BL], F32, tag="hr")
            nc.vector.tensor_max(hr[:, :], xr[:, :], xs[:, :])
            ps_p = ppr.tile([6, NSLOT * BL], F32, tag="psp")
            nc.tensor.matmul(ps_p[:, :], lhsT=wr2sb[:, :], rhs=hr[:, :],
                             start=True, stop=True)
            poses_sb = opool.tile([6, NSLOT * BL], F32, tag="poses")
            nc.scalar.activation(
                poses_sb[:, :], ps_p[:, :], ident,
                bias=br2sb[0:6, 0:1], scale=1.0,
            )
            nc.sync.dma_start(
                out=posesT[:],
                in_=poses_sb[:, :].rearrange("p (t b) -> p t b", t=NSLOT),
            )
            nc.sync.dma_start(out=hlastT[:], in_=zbuf[:, NSLOT - 1])

    nc.compile()
    return nc


_NC_CACHE = {}


def _get_nc(hsteps, wdt, fold_s, zero_bias):
    key = (tuple(float(x) for x in hsteps), wdt,
           None if fold_s is None else float(fold_s), bool(zero_bias))
    if key not in _NC_CACHE:
        _NC_CACHE[key] = _build_nc(list(key[0]), wdt, fold_s, zero_bias)
    return _NC_CACHE[key]


def prepare(fv, fi, ts, Wf0, bf0, Wf1, bf1, Wout, bout, Wr1, br1, Wr2, br2):
    """Build (nc, in_maps) for the current inputs/config."""
    ts = np.asarray(ts, dtype=np.float32)
    Wf0 = np.ascontiguousarray(np.asarray(Wf0, dtype=np.float32))
    Wf1 = np.ascontiguousarray(np.asarray(Wf1, dtype=np.float32))
    Wr1 = np.ascontiguousarray(np.asarray(Wr1, dtype=np.float32))
    Wr2 = np.ascontiguousarray(np.asarray(Wr2, dtype=np.float32))
    Wsub = np.ascontiguousarray(np.asarray(Wout)[:, ::C].astype(np.float32))
    bsub = np.ascontiguousarray(np.asarray(bout)[::C].astype(np.float32))
    bf0 = np.asarray(bf0, dtype=np.float32)
    bf1 = np.asarray(bf1, dtype=np.float32)
    br1 = np.asarray(br1, dtype=np.float32)
    br2 = np.asarray(br2, dtype=np.float32)

    s_all = (ts[:, 2] - ts[:, 1]).astype(np.float32)          # [B]
    eval_t = np.linspace(0.1, 1.0, NSLOT, dtype=np.float32)
    hsteps = (eval_t[1:] - eval_t[:-1]).astype(np.float32)    # [9]

    wdt = os.environ.get("POSECDE_WDT", "bf16")
    fold_s = float(s_all[0]) if np.all(s_all == s_all[0]) else None
    zero_bias = (os.environ.get("POSECDE_ZB", "1") == "1"
                 and not bf0.any() and not bf1.any() and not bsub.any())

    nc = _get_nc(hsteps, wdt, fold_s, zero_bias)

    npdt = {"f32": np.float32, "bf16": ml_dtypes.bfloat16, "f16": np.float16}[wdt]
    w0_in = Wf0.astype(npdt)
    w1_in = Wf1.astype(npdt)
    ws_in = Wsub.astype(npdt)
    wr1_in = Wr1.astype(npdt)

    def packed_aux(svec):
        a = np.zeros((128, NAUX), np.float32)
        a[:, 0:KC] = bf0.reshape(KC, 128).T
        a[:, KC:2 * KC] = bf1.reshape(KC, 128).T
        a[:, 2 * KC:3 * KC] = bsub.reshape(KC, 128).T
        a[:, 3 * KC] = br1
        a[0:6, 3 * KC + 1] = br2
        a[:, 3 * KC + 2:] = svec[None, :]
        return np.ascontiguousarray(a)

    shared = dict(w0=w0_in, w1=w1_in, ws=ws_in, wr1=wr1_in, wr2=Wr2)
    in_maps = []
    for i in range(N_CORES):
        m = dict(shared)
        m["aux"] = packed_aux(s_all[i * BL:(i + 1) * BL])
        in_maps.append(m)
    return nc, in_maps


def assemble(results):
    poses = np.empty((B, NSLOT, 6), np.float32)
    h_last = np.empty((B, H), np.float32)
    for i in range(N_CORES):
        pT = results[i]["posesT"]               # [6, 10, BL]
        poses[i * BL:(i + 1) * BL] = pT.transpose(2, 1, 0)
        hT = results[i]["hlastT"]               # [128, KC, BL]
        h_last[i * BL:(i + 1) * BL] = hT.transpose(2, 1, 0).reshape(BL, H)
    return poses, h_last


def kernel(**inputs):
    nc, in_maps = prepare(**inputs)
    res = run_bass_kernel_spmd(nc, in_maps, list(range(N_CORES))).results
    return assemble(res)


# revision 23
# speedup vs baseline: 1.0903x; 1.0903x over previous
"""Trainium2 Bass kernel for nn_PoseCDE.

Mathematical structure exploited (exact, input-independent):
  The CDE integrates over t in [0.1, 1.0], which lies entirely inside the
  FIRST segment of the rectilinear control path (segment grid spacing is 1,
  bucketize gives idx=0 for all eval times).  Segment 0's derivative is the
  time-advance knot: (ts[b,2]-ts[b,1], 0, ..., 0).  Hence
      f(t, z) = s_b * g(z)[:, :, 0]
  and only column 0 of each C-group of Wout matters:  Wsub = Wout[:, ::C].
  The 512 x 262656 matmul collapses to 512 x 512 (verified exact vs the
  reference for arbitrary inputs).

Device computation per core (data-parallel over batch, 8 rows per core):
  36 sequential 3-layer MLP evals (RK4, 9 steps) + linear regressor head.
  Activations are kept TRANSPOSED ([H on partitions, batch on free]) so
  weights are the PE-stationary operand and no on-chip transposes are
  needed; biases become per-partition operands.

Scheduling notes (trn2):
  - per-H-chunk PSUM groups + k-phase-major matmul order: each chunk's
    activation (DVE for early relu chunks, ACT otherwise) completes while
    the PE still streams the later chunks, so the next layer's k-phases
    find their inputs ready (software pipelining of the serial MLP chain).
  - weights in bf16 (FWL), fp32 PSUM accumulation, fp32 RK4 state.
  - uniform s and the uniform RK4 step h are folded into immediates.
"""

import os
import numpy as np
import ml_dtypes

import concourse.bass as bass
import concourse.bacc as bacc
import concourse.mybir as mybir
from concourse.tile import TileContext
from concourse.bass_utils import run_bass_kernel_spmd

N_CORES = 8
B = 64
BL = B // N_CORES          # batch rows per core
H = 512
C = H + 1
KC = H // 128              # H chunks (4)
NSLOT = 10                 # z0 + 9 RK4 states
F32 = mybir.dt.float32
BF16 = mybir.dt.bfloat16
F16 = mybir.dt.float16
NAUX = 3 * KC + 2 + BL     # packed aux columns: b0|b1|bs|br1|br2|svec


def _build_nc(hvals, wdt, fold_s, zero_bias):
    """wdt: "f32"|"bf16"|"f16"; fold_s: None or uniform-s float;
    zero_bias: True = MLP biases known to be zero."""
    nc = bacc.Bacc(None)
    WDT = {"f32": F32, "bf16": BF16, "f16": F16}[wdt]
    wdt_lp = wdt != "f32"

    w0 = nc.declare_dram_parameter("w0", [H, H], WDT, isOutput=False)
    w1 = nc.declare_dram_parameter("w1", [H, H], WDT, isOutput=False)
    ws = nc.declare_dram_parameter("ws", [H, H], WDT, isOutput=False)
    wr1 = nc.declare_dram_parameter("wr1", [H, 128], WDT, isOutput=False)
    wr2 = nc.declare_dram_parameter("wr2", [128, 6], F32, isOutput=False)
    aux = nc.declare_dram_parameter("aux", [128, NAUX], F32, isOutput=False)

    posesT = nc.declare_dram_parameter("posesT", [6, NSLOT, BL], F32, isOutput=True)
    hlastT = nc.declare_dram_parameter("hlastT", [128, KC, BL], F32, isOutput=True)

    relu = mybir.ActivationFunctionType.Relu
    tanh = mybir.ActivationFunctionType.Tanh
    ident = mybir.ActivationFunctionType.Identity
    mult = mybir.AluOpType.mult
    add = mybir.AluOpType.add
    amax = mybir.AluOpType.max

    def f32c(x):
        return float(np.float32(x))

    with TileContext(nc) as tc:
        with (
            tc.tile_pool(name="weights", bufs=1) as wpool,
            tc.tile_pool(name="state", bufs=1) as spool,
            tc.tile_pool(name="scratch", bufs=2) as scr,
            tc.tile_pool(name="psum", bufs=1, space="PSUM") as pp,
            tc.tile_pool(name="psum_r", bufs=1, space="PSUM") as ppr,
            tc.tile_pool(name="outs", bufs=1) as opool,
        ):
            # ---- load weights (parallel DMA dispatch across engines) ----
            w0sb = wpool.tile([128, KC, H], WDT, tag="w0")
            w1sb = wpool.tile([128, KC, H], WDT, tag="w1")
            wssb = wpool.tile([128, KC, H], WDT, tag="ws")
            wr1sb = wpool.tile([128, KC, 128], WDT, tag="wr1")
            wr2sb = wpool.tile([128, 6], F32, tag="wr2")
            auxsb = wpool.tile([128, NAUX], F32, tag="aux")
            nc.sync.dma_start(out=w0sb, in_=w0[:].rearrange("(k p) n -> p k n", p=128))
            nc.scalar.dma_start(out=w1sb, in_=w1[:].rearrange("(k p) n -> p k n", p=128))
            nc.gpsimd.dma_start(out=wssb, in_=ws[:].rearrange("(k p) n -> p k n", p=128))
            nc.gpsimd.dma_start(out=wr1sb, in_=wr1[:].rearrange("(k p) n -> p k n", p=128))
            nc.sync.dma_start(out=auxsb, in_=aux[:])
            nc.sync.dma_start(out=wr2sb, in_=wr2[:])

            b0sb = auxsb[:, 0:KC]
            b1sb = auxsb[:, KC:2 * KC]
            bssb = auxsb[:, 2 * KC:3 * KC]
            br1sb = auxsb[:, 3 * KC:3 * KC + 1]
            br2sb = auxsb[:, 3 * KC + 1:3 * KC + 2]   # first 6 partitions valid
            s_sb = auxsb[:, 3 * KC + 2:3 * KC + 2 + BL]

            def s_ap():
                # broadcast s_sb [128, BL] over the KC free dim
                t = s_sb
                return bass.AP(
                    tensor=t.tensor, offset=t.offset,
                    ap=[t.ap[0], [0, KC], t.ap[1]],
                )

            # ---- absorb input-DMA sems into engine vector clocks ----
            # (the S3_LW weight-load struct only fits ONE sync wait; these
            # 1x1 matmuls make every later PE inst see the DMAs as done)
            dummy_ps = ppr.tile([1, 1], F32, name="dps", tag="psr")

            def absorb(wtile):
                sl = wtile[:, 0, 0:1] if len(wtile.shape) == 3 else wtile[:, 0:1]
                nc.tensor.matmul(dummy_ps[:, :], lhsT=sl, rhs=sl,
                                 start=True, stop=True)

            absorb(w0sb)
            _pending_absorbs = [w1sb, wssb, wr1sb, wr2sb]

            # ---- state buffers ----
            zbuf = spool.tile([128, NSLOT, KC, BL], F32, tag="zbuf")
            nc.vector.memset(zbuf[:, 0], 0.0)
            if wdt_lp:
                zbuf_m = spool.tile([128, NSLOT, KC, BL], WDT, tag="zbufm")
                nc.vector.memset(zbuf_m[:, 0], 0.0)

            # group width: halves when biases are zero (one op can cover
            # two H-chunks), per-chunk otherwise (per-partition bias APs)
            groups = [(0, 2), (2, 2)] if zero_bias else [(0, 1), (1, 1), (2, 1), (3, 1)]
            ps_bufs = 2 if zero_bias else 1

            def mlp_layer(in_t, w_t, b_t, func, out_t):
                """out_t[128,KC,BL] = func(matmul(in_t) + bias).  Relu
                consumers go to DVE (cheap, ~170ns), tanh to ACT; group
                granularity balances op count vs pipelining."""
                is_relu = func is relu
                pss = [pp.tile([128, gw, BL], F32, name=f"psg{gi}",
                               tag=f"psg{gi}", bufs=ps_bufs)
                       for gi, (m0, gw) in enumerate(groups)]
                for kph in (range(0, KC // 2), range(KC // 2, KC)):
                    for gi, (m0, gw) in enumerate(groups):
                        for mm in range(gw):
                            for k in kph:
                                nc.tensor.matmul(
                                    pss[gi][:, mm],
                                    lhsT=w_t[:, k, bass.ts(m0 + mm, 128)],
                                    rhs=in_t[:, k, :],
                                    start=(k == 0),
                                    stop=(k == KC - 1),
                                    skip_group_check=True,
                                )
                for gi, (m0, gw) in enumerate(groups):
                    ps = pss[gi]
                    sl = slice(m0, m0 + gw)
                    if is_relu:
                        if zero_bias:
                            nc.vector.tensor_scalar(
                                out_t[:, sl], ps[:], 0.0, None, op0=amax)
                        else:
                            nc.vector.tensor_scalar(
                                out_t[:, sl], ps[:],
                                b_t[:, m0:m0 + 1], 0.0, op0=add, op1=amax)
                    else:
                        bias = 0.0 if zero_bias else b_t[:, m0:m0 + 1]
                        nc.scalar.activation(
                            out_t[:, sl], ps[:], func, bias=bias, scale=1.0)

            def mlp(in_t, utag):
                h1 = scr.tile([128, KC, BL], WDT, tag="h1")
                h2 = scr.tile([128, KC, BL], WDT, tag="h2")
                u = scr.tile([128, KC, BL], F32, tag=utag)
                mlp_layer(in_t, w0sb, b0sb, relu, h1)
                if _pending_absorbs:
                    absorb(_pending_absorbs.pop(0))
                mlp_layer(h1, w1sb, b1sb, relu, h2)
                if _pending_absorbs:
                    absorb(_pending_absorbs.pop(0))
                mlp_layer(h2, wssb, bssb, tanh, u)
                while _pending_absorbs:
                    absorb(_pending_absorbs.pop(0))
                return u

            # ---- RK4 ----
            for t in range(9):
                hf = f32c(hvals[t])
                half_h = f32c(np.float32(0.5) * np.float32(hf))
                h6 = f32c(np.float32(hf) / np.float32(6.0))
                z_t = zbuf[:, t]

                if wdt_lp:
                    if fold_s is None and t > 0:
                        zt_m = scr.tile([128, KC, BL], WDT, tag="ztm")
                        nc.vector.tensor_copy(zt_m[:], z_t)
                        ev1_in = zt_m
                    else:
                        ev1_in = zbuf_m[:, t]
                else:
                    ev1_in = z_t

                if fold_s is not None:
                    sv = np.float32(fold_s)
                    c1 = f32c(np.float32(half_h) * sv)   # 0.5*h*s
                    c2 = f32c(np.float32(hf) * sv)       # h*s
                    c3 = f32c(np.float32(h6) * sv)       # (h/6)*s

                    def chunk_stt(dst, src0, scal, src1):
                        for (m0, gw) in groups:
                            sl = slice(m0, m0 + gw)
                            nc.vector.scalar_tensor_tensor(
                                out=dst[:, sl], in0=src0[:, sl], scalar=scal,
                                in1=src1[:, sl], op0=mult, op1=add)

                    u1 = mlp(ev1_in, "u1")
                    zin1 = scr.tile([128, KC, BL], WDT, tag="zin")
                    chunk_stt(zin1, u1, c1, z_t)
                    u2 = mlp(zin1, "u2")
                    zin2 = scr.tile([128, KC, BL], WDT, tag="zin")
                    chunk_stt(zin2, u2, c1, z_t)
                    # acc = u1 + 2*u2 (+2*u3) built mid-step, off-chain
                    acc = scr.tile([128, KC, BL], F32, tag="acc")
                    nc.vector.scalar_tensor_tensor(
                        out=acc[:], in0=u2[:], scalar=2.0, in1=u1[:],
                        op0=mult, op1=add)
                    u3 = mlp(zin2, "u3")
                    zin3 = scr.tile([128, KC, BL], WDT, tag="zin")
                    chunk_stt(zin3, u3, c2, z_t)
                    nc.vector.scalar_tensor_tensor(
                        out=acc[:], in0=u3[:], scalar=2.0, in1=acc[:],
                        op0=mult, op1=add)
                    # w = z + c3*acc  (so z' = w + c3*u4: kills one serial
                    # DVE hop at the step boundary)
                    wv = scr.tile([128, KC, BL], F32, tag="wv")
                    nc.vector.scalar_tensor_tensor(
                        out=wv[:], in0=acc[:], scalar=c3, in1=z_t[:],
                        op0=mult, op1=add)
                    u4 = mlp(zin3, "u4")
                    if wdt_lp:
                        chunk_stt(zbuf_m[:, t + 1], u4, c3, wv)
                    nc.vector.scalar_tensor_tensor(
                        out=zbuf[:, t + 1], in0=u4[:], scalar=c3, in1=wv[:],
                        op0=mult, op1=add)
                else:
                    u1 = mlp(ev1_in, "u1")
                    k1 = scr.tile([128, KC, BL], F32, tag="k1")
                    nc.vector.tensor_mul(k1[:], u1[:], s_ap())
                    zin1 = scr.tile([128, KC, BL], WDT, tag="zin")
                    nc.vector.scalar_tensor_tensor(
                        out=zin1[:], in0=k1[:], scalar=half_h, in1=z_t,
                        op0=mult, op1=add)
                    u2 = mlp(zin1, "u2")
                    k2 = scr.tile([128, KC, BL], F32, tag="k2")
                    nc.vector.tensor_mul(k2[:], u2[:], s_ap())
                    zin2 = scr.tile([128, KC, BL], WDT, tag="zin")
                    nc.vector.scalar_tensor_tensor(
                        out=zin2[:], in0=k2[:], scalar=half_h, in1=z_t,
                        op0=mult, op1=add)
                    u3 = mlp(zin2, "u3")
                    k3 = scr.tile([128, KC, BL], F32, tag="k3")
                    nc.vector.tensor_mul(k3[:], u3[:], s_ap())
                    zin3 = scr.tile([128, KC, BL], WDT, tag="zin")
                    nc.vector.scalar_tensor_tensor(
                        out=zin3[:], in0=k3[:], scalar=hf, in1=z_t,
                        op0=mult, op1=add)
                    u4 = mlp(zin3, "u4")
                    k4 = scr.tile([128, KC, BL], F32, tag="k4")
                    nc.vector.tensor_mul(k4[:], u4[:], s_ap())
                    acc = scr.tile([128, KC, BL], F32, tag="acc")
                    nc.vector.scalar_tensor_tensor(
                        out=acc[:], in0=k2[:], scalar=2.0, in1=k1[:],
                        op0=mult, op1=add)
                    nc.vector.scalar_tensor_tensor(
                        out=acc[:], in0=k3[:], scalar=2.0, in1=acc[:],
                        op0=mult, op1=add)
                    nc.vector.tensor_add(acc[:], acc[:], k4[:])
                    nc.vector.scalar_tensor_tensor(
                        out=zbuf[:, t + 1], in0=acc[:], scalar=h6, in1=z_t,
                        op0=mult, op1=add)

            # ---- regressor head ----
            if wdt_lp and fold_s is not None:
                zreg = zbuf_m
            elif wdt_lp:
                zreg = spool.tile([128, NSLOT, KC, BL], WDT, tag="zregm")
                nc.vector.tensor_copy(zreg[:], zbuf[:])
            else:
                zreg = zbuf
            ps_r = ppr.tile([128, NSLOT * BL], F32, tag="psr2")
            for k in range(KC):
                nc.tensor.matmul(
                    ps_r[:, :],
                    lhsT=wr1sb[:, k, :],
                    rhs=zreg[:, :, k, :],
                    start=(k == 0),
                    stop=(k == KC - 1),
                )
            # leaky_relu(x) = max(x, 0.1*x)  (exact for slope < 1)
            xr = opool.tile([128, NSLOT * BL], F32, tag="xr")
            nc.scalar.activation(
                xr[:, :], ps_r[:, :], ident,
                bias=br1sb, scale=1.0,
            )
            xs = opool.tile([128, NSLOT * BL], F32, tag="xs")
            nc.vector.tensor_scalar_mul(xs[:, :], xr[:, :], 0.1)
            hr = opool.tile([128, NSLOT * BL], F32, tag="hr")
            nc.vector.tensor_max(hr[:, :], xr[:, :], xs[:, :])
            ps_p = ppr.tile([6, NSLOT * BL], F32, tag="psp")
            nc.tensor.matmul(ps_p[:, :], lhsT=wr2sb[:, :], rhs=hr[:, :],
                             start=True, stop=True)
            poses_sb = opool.tile([6, NSLOT * BL], F32, tag="poses")
            nc.scalar.activation(
                poses_sb[:, :], ps_p[:, :], ident,
                bias=br2sb[0:6, 0:1], scale=1.0,
            )
            nc.sync.dma_start(
                out=posesT[:],
                in_=poses_sb[:, :].rearrange("p (t b) -> p t b", t=NSLOT),
            )
            nc.sync.dma_start(out=hlastT[:], in_=zbuf[:, NSLOT - 1])

    nc.compile()
    return nc


_NC_CACHE = {}


def _get_nc(hsteps, wdt, fold_s, zero_bias):
    key = (tuple(float(x) for x in hsteps), wdt,
           None if fold_s is None else float(fold_s), bool(zero_bias))
    if key not in _NC_CACHE:
        _NC_CACHE[key] = _build_nc(list(key[0]), wdt, fold_s, zero_bias)
    return _NC_CACHE[key]


def prepare(fv, fi, ts, Wf0, bf0, Wf1, bf1, Wout, bout, Wr1, br1, Wr2, br2):
    """Build (nc, in_maps) for the current inputs/config."""
    ts = np.asarray(ts, dtype=np.float32)
    Wf0 = np.ascontiguousarray(np.asarray(Wf0, dtype=np.float32))
    Wf1 = np.ascontiguousarray(np.asarray(Wf1, dtype=np.float32))
    Wr1 = np.ascontiguousarray(np.asarray(Wr1, dtype=np.float32))
    Wr2 = np.ascontiguousarray(np.asarray(Wr2, dtype=np.float32))
    Wsub = np.ascontiguousarray(np.asarray(Wout)[:, ::C].astype(np.float32))
    bsub = np.ascontiguousarray(np.asarray(bout)[::C].astype(np.float32))
    bf0 = np.asarray(bf0, dtype=np.float32)
    bf1 = np.asarray(bf1, dtype=np.float32)
    br1 = np.asarray(br1, dtype=np.float32)
    br2 = np.asarray(br2, dtype=np.float32)

    s_all = (ts[:, 2] - ts[:, 1]).astype(np.float32)          # [B]
    eval_t = np.linspace(0.1, 1.0, NSLOT, dtype=np.float32)
    hsteps = (eval_t[1:] - eval_t[:-1]).astype(np.float32)    # [9]

    wdt = os.environ.get("POSECDE_WDT", "bf16")
    fold_s = float(s_all[0]) if np.all(s_all == s_all[0]) else None
    zero_bias = (os.environ.get("POSECDE_ZB", "1") == "1"
                 and not bf0.any() and not bf1.any() and not bsub.any())

    nc = _get_nc(hsteps, wdt, fold_s, zero_bias)

    npdt = {"f32": np.float32, "bf16": ml_dtypes.bfloat16, "f16": np.float16}[wdt]
    w0_in = Wf0.astype(npdt)
    w1_in = Wf1.astype(npdt)
    ws_in = Wsub.astype(npdt)
    wr1_in = Wr1.astype(npdt)

    def packed_aux(svec):
        a = np.zeros((128, NAUX), np.float32)
        a[:, 0:KC] = bf0.reshape(KC, 128).T
        a[:, KC:2 * KC] = bf1.reshape(KC, 128).T
        a[:, 2 * KC:3 * KC] = bsub.reshape(KC, 128).T
        a[:, 3 * KC] = br1
        a[0:6, 3 * KC + 1] = br2
        a[:, 3 * KC + 2:] = svec[None, :]
        return np.ascontiguousarray(a)

    shared = dict(w0=w0_in, w1=w1_in, ws=ws_in, wr1=wr1_in, wr2=Wr2)
    in_maps = []
    for i in range(N_CORES):
        m = dict(shared)
        m["aux"] = packed_aux(s_all[i * BL:(i + 1) * BL])
        in_maps.append(m)
    return nc, in_maps


def assemble(results):
    poses = np.empty((B, NSLOT, 6), np.float32)
    h_last = np.empty((B, H), np.float32)
    for i in range(N_CORES):
        pT = results[i]["posesT"]               # [6, 10, BL]
        poses[i * BL:(i + 1) * BL] = pT.transpose(2, 1, 0)
        hT = results[i]["hlastT"]               # [128, KC, BL]
        h_last[i * BL:(i + 1) * BL] = hT.transpose(2, 1, 0).reshape(BL, H)
    return poses, h_last


def kernel(**inputs):
    nc, in_maps = prepare(**inputs)
    res = run_bass_kernel_spmd(nc, in_maps, list(range(N_CORES))).results
    return assemble(res)
